# revision 31
# baseline (speedup 1.0000x reference)
"""Sparse transposed-conv (27-tap gather-GEMM) + BatchNorm + LeakyReLU on 8 TRN2 cores.

Strategy (component-sharded, SBUF-resident accumulator, bf16 compute):
  - Host: connected components of the neighbor graph are balanced across
    cores (largest-first bin packing) and made contiguous by a global
    reorder; each core owns 25000 voxels plus a small halo of
    cross-boundary source rows, so every gather is core-local
    (< 32768 rows -> single int16 window, one dma_gather per tap).
  - Device, per core: per tap dma_gather fp32 x rows -> PE-transpose
    128x128 chunks to channel-major (cast bf16) -> one bf16 matmul per
    512 columns against a block-diag [[Wk,0],[0,Wk]] stationary ->
    PE-transpose back -> dma_scatter_add into a bf16 SBUF-resident h
    accumulator (parity-split CCE add; v -> parity (v>>7)&1,
    partition v%128, group v>>8). Pads gather a guaranteed-zero x row
    and scatter into trash groups >= TR0.
  - h is split over THREE accumulator pairs (even taps / odd taps /
    center) merged by DVE at the end: scatter-adds to one tile pair form
    a WAW chain that serializes on HW, so independent pairs double the
    scatter overlap. Center accumulates with strided DVE adds (no
    scatter). Gathers/scatters spread over 4 SWDGE queues with a
    queue-aware patch of Tile's DMASW lane rotation.
  - BN mean: per-channel sums fall out of the matmul pipeline for free
    (accum_out on the PSUM->SBUF copy, folded with a [I;I] matmul).
    BN var: ACT squares h, DVE reduces per channel, ones-matmul folds
    partitions; [64,2] AllReduce over 8 cores; y = lrelu(h*s+b) applied
    chunkwise (DVE affine + ACT lrelu) and DMA'd out per chunk.
"""
import numpy as np

import concourse.bass as bass
import concourse.mybir as mybir
import concourse.bacc as bacc
import concourse.tile as tile
from concourse import bass_utils
from concourse.masks import make_identity

# Tile assigns SWDGE completion-sem lanes round-robin over Pool DMAs,
# assuming FIFO completion within a lane. With multiple SWDGE queues that
# assumption breaks (cross-queue reordering on one counting lane lets a
# consumer's wait_ge pass early). Make the lane a function of the queue:
# 8 lanes / 4 queues = 2 lanes per queue, each lane fed by exactly one
# queue (FIFO ring) so counting waits stay sound.
import concourse.tile_sem_assignment as _tsa
from concourse.tile_scheduler import DMAInst as _DMAInst
import concourse.bass_isa as _bass_isa

_orig_assign_tick = _tsa.TileClockTick._assign_tick


def _qaware_assign_tick(self, inst):
    if (isinstance(inst, _DMAInst)
            and not isinstance(inst, _bass_isa.UserSyncedRemoteDMADescs)
            and inst.engine == mybir.EngineType.Pool):
        q = int(getattr(inst, "queue_num", 0) or 0)
        ctr = getattr(self, "_qlane_ctr", None)
        if ctr is None:
            ctr = self._qlane_ctr = {}
        self.next_sw_dma_idx = q * 2 + ctr.get(q, 0) % 2
        ctr[q] = ctr.get(q, 0) + 1
    return _orig_assign_tick(self, inst)


if _tsa.TileClockTick._assign_tick is not _qaware_assign_tick:
    _tsa.TileClockTick._assign_tick = _qaware_assign_tick

N = 200000
C = 8
V = N // C          # 25000 voxels per core
D = 64
K = 27
KC = 13             # center tap (identity map)
GRP = 100           # h groups per parity buffer (covers v < 25600)
NSTAT = 98          # groups included in BN stats / y (v < 25088)
TR0 = 25088         # trash dst base (groups 98, 99)
EPS = 1e-5
NEG = 0.01
F32 = mybir.dt.float32
BF16 = mybir.dt.bfloat16
I16 = mybir.dt.int16


def _r128(n):
    return (n + 127) & ~127


def _r256(n):
    return (n + 255) & ~255


def _pack16(slab, col0, vals):
    """Place index list (len mult of 16) at int16-slab columns col0.., wrapped
    [i%16, i//16] and replicated to all 8 Q7 core partition groups."""
    w = vals.reshape(-1, 16).T
    L16 = w.shape[1]
    for r in range(8):
        slab[r * 16:(r + 1) * 16, col0:col0 + L16] = w
    return col0 + L16


def _components(nbr):
    import scipy.sparse as sp
    import scipy.sparse.csgraph as csg
    Kk, n = nbr.shape
    src = np.repeat(np.arange(n), Kk)
    dst = nbr.T.ravel()
    m = dst >= 0
    A = sp.coo_matrix((np.ones(m.sum(), np.int8), (src[m], dst[m])),
                      shape=(n, n))
    _, lab = csg.connected_components(A, directed=False)
    return lab


def _prep_host(nbr):
    """Component-contiguous reorder + per-core compacted local kernel maps."""
    nbr = np.asarray(nbr, np.int64)
    lab = _components(nbr)
    # balance whole components across cores (largest-first, least-loaded)
    import heapq
    sizes = np.bincount(lab)
    assign = np.empty(len(sizes), np.int64)
    heap = [(0, c) for c in range(C)]
    heapq.heapify(heap)
    for comp in np.argsort(sizes)[::-1]:
        load, c = heapq.heappop(heap)
        assign[comp] = c
        heapq.heappush(heap, (load + int(sizes[comp]), c))
    perm = np.lexsort((lab, assign[lab]))      # rank -> orig
    rank = np.empty(N, np.int64)
    rank[perm] = np.arange(N)                  # orig -> rank

    # per-(core, tap) local (src, dst) pair lists
    ks = [k for k in range(K) if k != KC]
    nbr_r = np.where(nbr >= 0, rank[np.clip(nbr, 0, None)], -1)  # in rank space
    nbr_r = nbr_r[:, perm]                     # column r: dst rank r
    lists = {}
    halos = []
    for c in range(C):
        lo, hi = c * V, (c + 1) * V
        ext = set()
        per_k = {}
        for k in ks:
            srcs = nbr_r[k, lo:hi]
            valid = np.nonzero(srcs >= 0)[0]
            s = srcs[valid]
            per_k[k] = (s, valid)
            out = s[(s < lo) | (s >= hi)]
            ext.update(out.tolist())
        halo = np.sort(np.fromiter(ext, np.int64, len(ext)))
        halos.append(halo)
        for k in ks:
            s, valid = per_k[k]
            inr = (s >= lo) & (s < hi)
            loc = np.where(inr, s - lo, V + np.searchsorted(halo, s))
            o = np.argsort(loc, kind="stable")
            lists[(c, k)] = (loc[o].astype(np.int16), valid[o].astype(np.int16))

    XL = _r128(V + max(len(h) for h in halos) + 1)
    assert XL <= 32768, XL

    NK = {k: max(256, _r256(max(len(lists[(c, k)][0]) for c in range(C))))
          for k in ks}
    GT = sum(NK.values())
    max_npad = max(NK[k] - len(lists[(c, k)][0])
                   for k in ks for c in range(C))
    grp = max(GRP, -(-(TR0 + max_npad) // 256))
    gslab = np.zeros((C, 128, GT // 16), np.int16)
    sslab = np.zeros((C, 128, GT // 16), np.int16)
    plan = []
    koff = 0
    for k in ks:
        plan.append((k, NK[k], koff))
        for c in range(C):
            g, s = lists[(c, k)]
            L = NK[k]
            gp = np.full(L, XL - 1, np.int16)
            gp[:len(g)] = g
            sp_ = np.empty(L, np.int16)
            sp_[:len(s)] = s
            npad = L - len(s)
            assert npad <= 256 * grp - TR0, (npad, grp)
            if npad:
                sp_[len(s):] = TR0 + np.arange(npad, dtype=np.int16)
            _pack16(gslab[c], koff // 16, gp)
            _pack16(sslab[c], koff // 16, sp_)
        koff += NK[k]

    xsel = []
    for c in range(C):
        sel = np.concatenate([perm[c * V:(c + 1) * V], perm[halos[c]]])
        xsel.append(sel)
    return plan, GT, XL, grp, gslab, sslab, xsel, perm


def _build_program(plan, GT, XL, grp):
    nc = bacc.Bacc("TRN2", target_bir_lowering=False, debug=False,
                   num_devices=C, num_swdge_queues=4)

    xc_d = nc.dram_tensor("xc_d", [XL, D], F32, kind="ExternalInput")
    xcb_d = nc.dram_tensor("xcb_d", [25088, D], BF16, kind="ExternalInput")
    W2_d = nc.dram_tensor("W2_d", [128, K * 128], BF16, kind="ExternalInput")
    gam_d = nc.dram_tensor("gam_d", [1, D], F32, kind="ExternalInput")
    bet_d = nc.dram_tensor("bet_d", [1, D], F32, kind="ExternalInput")
    gi_d = nc.dram_tensor("gi_d", [128, GT // 16], I16, kind="ExternalInput")
    si_d = nc.dram_tensor("si_d", [128, GT // 16], I16, kind="ExternalInput")
    y_d = nc.dram_tensor("y_d", [V, D], F32, kind="ExternalOutput")
    import os as _os
    _dbg = _os.environ.get("KERNEL_DEBUG_H")
    if _dbg:
        hdbg_d = nc.dram_tensor("hdbg_d", [2, 128, grp * D], BF16,
                                kind="ExternalOutput")

    NKmax = max(nk for _, nk, _ in plan)

    with tile.TileContext(nc) as tc:
        with tc.tile_pool(name="sb", bufs=1) as sb, \
             tc.tile_pool(name="io", bufs=3) as io, \
             tc.tile_pool(name="io2", bufs=4) as io2, \
             tc.tile_pool(name="ps", bufs=2, space="PSUM") as ps, \
             tc.tile_pool(name="dram", bufs=1, space="DRAM") as dram:

            ident = sb.tile([128, 128], F32)
            make_identity(nc, ident[:])
            ident_b = sb.tile([128, 128], BF16)
            nc.vector.tensor_copy(ident_b[:], ident[:])
            gi_t = sb.tile([128, GT // 16], I16)
            nc.sync.dma_start(gi_t[:], gi_d[:, :])
            si_t = sb.tile([128, GT // 16], I16)
            nc.sync.dma_start(si_t[:], si_d[:, :])

            W2 = sb.tile([128, K * 128], BF16)
            nc.sync.dma_start(W2[:], W2_d[:, :])

            hs_e = sb.tile([128, grp, D], BF16)
            hs_o = sb.tile([128, grp, D], BF16)
            hs_e2 = sb.tile([128, grp, D], BF16)
            hs_o2 = sb.tile([128, grp, D], BF16)
            hs_ec = sb.tile([128, grp, D], BF16)
            hs_oc = sb.tile([128, grp, D], BF16)
            hs_e3 = sb.tile([128, grp, D], BF16)
            hs_o3 = sb.tile([128, grp, D], BF16)
            nc.gpsimd.memset(hs_e3[:], 0.0)
            nc.gpsimd.memset(hs_o3[:], 0.0)
            nc.gpsimd.memset(hs_e[:], 0.0)
            nc.gpsimd.memset(hs_o[:], 0.0)
            nc.gpsimd.memset(hs_e2[:], 0.0)
            nc.gpsimd.memset(hs_o2[:], 0.0)
            nc.gpsimd.memset(hs_ec[:], 0.0)
            nc.gpsimd.memset(hs_oc[:], 0.0)

            acc = sb.tile([128, 128], F32)
            nc.gpsimd.memset(acc[:], 0.0)
            acc_n = [0]

            def mid(gsrc, kk, cols, sout, scol, bf=False):
                """gsrc[:, cols] (2-voxel 128-col chunks) -> transpose -> MM
                W2[kk] -> transpose back -> sout[:, scol:scol+128*len(cols)].
                The PSUM->SBUF copy also bank-sums columns into acc (per
                (slot,ch) partition) for the BN mean."""
                gw = len(cols) * 128
                pa = ps.tile([128, 512], BF16 if bf else F32, tag="psA",
                             space="PSUM")
                idt = ident_b if bf else ident
                for j, cj in enumerate(cols):
                    nc.tensor.transpose(
                        out=pa[:, j * 128:(j + 1) * 128],
                        in_=gsrc[:, cj * 128:(cj + 1) * 128], identity=idt[:])
                ct = io.tile([128, 512], BF16, tag="ct")
                nc.scalar.activation(ct[:, :gw], pa[:, :gw],
                                     mybir.ActivationFunctionType.Copy,
                                     bias=0.0)
                pb = ps.tile([128, 512], F32, tag="psB", space="PSUM")
                nc.tensor.matmul(out=pb[:, :gw],
                                 lhsT=W2[:, kk * 128:(kk + 1) * 128],
                                 rhs=ct[:, :gw], start=True, stop=True)
                hb = io.tile([128, 512], BF16, tag="hb")
                ai = acc_n[0]
                acc_n[0] += 1
                nc.vector.tensor_scalar(out=hb[:, :gw], in0=pb[:, :gw],
                                        scalar1=0.0, scalar2=0.0,
                                        op0=mybir.AluOpType.add,
                                        op1=mybir.AluOpType.add,
                                        accum_out=acc[:, ai:ai + 1])
                pc = ps.tile([128, 512], BF16, tag="psC", space="PSUM")
                for j in range(len(cols)):
                    nc.tensor.transpose(
                        out=pc[:, j * 128:(j + 1) * 128],
                        in_=hb[:, j * 128:(j + 1) * 128], identity=ident_b[:])
                nc.scalar.activation(sout[:, scol:scol + gw], pc[:, :gw],
                                     mybir.ActivationFunctionType.Copy,
                                     bias=0.0)

            # ---- center tap: dense, strided-copied into h ----
            def center_group(g):
                rows = 512 if g == 24 else 1024
                nm = rows // 128
                xg = io.tile([128, 8, D], BF16, tag="xg")
                xv = xcb_d[g * 1024:g * 1024 + rows, :].rearrange(
                    "(m p) c -> p m c", p=128)
                nc.sync.dma_start(xg[:, 0:nm, :], xv)
                xgf = xg[:].rearrange("p m d -> p (m d)")
                cs = io.tile([128, 8, D], BF16, tag="cs")
                csf = cs[:].rearrange("p m d -> p (m d)")
                mid(xgf, KC, list(range(nm // 2)), csf, 0, bf=True)
                def acc(dst, src):
                    nc.vector.tensor_tensor(out=dst, in0=dst, in1=src,
                                            op=mybir.AluOpType.add)
                if g < 24:
                    acc(hs_ec[:, 4 * g:4 * g + 4, :], cs[:, 0:8:2, :])
                    acc(hs_oc[:, 4 * g:4 * g + 4, :], cs[:, 1:8:2, :])
                else:
                    acc(hs_ec[:, 96:98, :], cs[:, 0:4:2, :])
                    acc(hs_oc[:, 96:97, :], cs[:, 1:2, :])
                    acc(hs_oc[0:40, 97:98, :], cs[0:40, 3:4, :])

            # ---- sparse taps (depth-2 software pipeline) ----
            pdma_ctr = [0]

            def nextq():
                q = pdma_ctr[0] % 4
                pdma_ctr[0] += 1
                return q

            def tap_gather(ki):
                k, NKk, koff = plan[ki]
                gb = io2.tile([128, NKmax // 128, D], F32, tag="gb")
                nc.gpsimd.dma_gather(
                    out_ap=gb[:, 0:NKk // 128, :],
                    in_ap=xc_d[:, :],
                    idxs_ap=gi_t[:, koff // 16:(koff + NKk) // 16],
                    num_idxs=NKk, num_idxs_reg=NKk, elem_size=D,
                    single_packet=False, queue_num=nextq())
                return gb

            def tap_compute(ki, gb):
                k, NKk, koff = plan[ki]
                gbf = gb[:].rearrange("p m d -> p (m d)")
                sk = io2.tile([128, NKmax // 128, D], BF16, tag="sk")
                skf = sk[:].rearrange("p m d -> p (m d)")
                nch = NKk // 256
                for c0 in range(0, nch, 4):
                    cols = list(range(c0, min(c0 + 4, nch)))
                    mid(gbf, k, cols, skf, c0 * 128)
                he, ho = ((hs_e, hs_o), (hs_e2, hs_o2),
                          (hs_e3, hs_o3))[ki % 3]
                nc.gpsimd.dma_scatter_add(
                    out_ap=he[:], out_ap_other=ho[:],
                    in_ap=sk[:, 0:NKk // 128, :],
                    idxs_ap=si_t[:, koff // 16:(koff + NKk) // 16],
                    num_idxs=NKk, num_idxs_reg=NKk, elem_size=D,
                    single_packet=False, queue_num=nextq(),
                    sbuf_tokens_per_rank=128, parity_reg=0)

            pend = []
            cg = 0
            for ki in range(len(plan)):
                pend.append((ki, tap_gather(ki)))
                if cg < 25:
                    center_group(cg)
                    cg += 1
                if len(pend) > 3:
                    kj, gbj = pend.pop(0)
                    tap_compute(kj, gbj)
            while cg < 25:
                center_group(cg)
                cg += 1
            for kj, gbj in pend:
                tap_compute(kj, gbj)

            if _dbg:
                nc.sync.dma_start(hdbg_d[0, :, :],
                                  hs_e[:].rearrange("p g d -> p (g d)"))
                nc.sync.dma_start(hdbg_d[1, :, :],
                                  hs_o[:].rearrange("p g d -> p (g d)"))
            # merge the extra accumulator pairs
            for dst_t, src_t in ((hs_e, hs_e2), (hs_o, hs_o2),
                                 (hs_e, hs_e3), (hs_o, hs_o3),
                                 (hs_e, hs_ec), (hs_o, hs_oc)):
                nc.vector.tensor_tensor(
                    out=dst_t[:].rearrange("p g d -> p (g d)"),
                    in0=dst_t[:].rearrange("p g d -> p (g d)"),
                    in1=src_t[:].rearrange("p g d -> p (g d)"),
                    op=mybir.AluOpType.add)
            # ---- BN stats: mean from acc, sumsq from SBUF h ----
            q_pch = sb.tile([128, D], F32)
            q_tmp = sb.tile([128, D], F32)
            for pi, hs in ((0, hs_e), (1, hs_o)):
                qo = q_pch if pi == 0 else q_tmp
                sq_t = hs_e2 if pi == 0 else hs_o2
                sqf = sq_t[:, 0:NSTAT, :].rearrange("p g d -> p (g d)")
                hf = hs[:, 0:NSTAT, :].rearrange("p g d -> p (g d)")
                nc.scalar.activation(sqf, hf,
                                     mybir.ActivationFunctionType.Square,
                                     bias=0.0, scale=1.0)
                qview = sq_t[:, 0:NSTAT, :].rearrange("p g d -> p d g")
                nc.vector.tensor_reduce(out=qo[:], in_=qview,
                                        axis=mybir.AxisListType.X,
                                        op=mybir.AluOpType.add)
            nc.vector.tensor_tensor(out=q_pch[:], in0=q_pch[:], in1=q_tmp[:],
                                    op=mybir.AluOpType.add)
            acc_red = sb.tile([128, 1], F32)
            nc.vector.tensor_reduce(out=acc_red[:], in_=acc[:],
                                    axis=mybir.AxisListType.X,
                                    op=mybir.AluOpType.add)
            fold = sb.tile([128, D], F32)
            nc.vector.tensor_copy(fold[0:64, :], ident[0:64, 0:64])
            nc.vector.tensor_copy(fold[64:128, :], ident[0:64, 0:64])
            ones1 = sb.tile([128, 1], F32)
            nc.gpsimd.memset(ones1[:], 1.0)
            pS = ps.tile([64, 2], F32, tag="psS", space="PSUM")
            nc.tensor.matmul(out=pS[:, 0:1], lhsT=fold[:], rhs=acc_red[:],
                             start=True, stop=True)
            nc.tensor.matmul(out=pS[:, 1:2], lhsT=q_pch[:], rhs=ones1[:],
                             start=True, stop=True)
            sq64 = sb.tile([64, 2], F32)
            nc.vector.tensor_copy(sq64[:], pS[:])

            cc_in = dram.tile([64, 2], F32)
            cc_out = dram.tile([64, 2], F32)
            nc.sync.dma_start(cc_in[:], sq64[:])
            import os as _os
            if _os.environ.get("KERNEL_SIM_NOCC"):
                nc.sync.dma_start(cc_out[:], cc_in[:])
            else:
                nc.gpsimd.collective_compute(
                    "AllReduce", mybir.AluOpType.add,
                    replica_groups=[list(range(C))],
                    ins=[cc_in.opt()], outs=[cc_out.opt()])
            g2 = sb.tile([64, 2], F32)
            nc.sync.dma_start(g2[:], cc_out[:])

            # per-channel BN coefficients (channel-major on 64 partitions)
            me = sb.tile([64, 2], F32)
            nc.vector.tensor_scalar_mul(me[:], g2[:], 1.0 / N)  # [mean, Eh2]
            v1 = sb.tile([64, 1], F32)
            nc.vector.tensor_tensor(out=v1[:], in0=me[:, 0:1], in1=me[:, 0:1],
                                    op=mybir.AluOpType.mult)
            nc.vector.tensor_tensor(out=v1[:], in0=me[:, 1:2], in1=v1[:],
                                    op=mybir.AluOpType.subtract)
            eps_t = sb.tile([64, 1], F32)
            nc.gpsimd.memset(eps_t[:], EPS)
            std = sb.tile([64, 1], F32)
            nc.scalar.activation(std[:], v1[:],
                                 mybir.ActivationFunctionType.Sqrt,
                                 bias=eps_t[:])
            rin = sb.tile([64, 1], F32)
            nc.vector.reciprocal(rin[:], std[:])
            gam = sb.tile([64, 1], F32)
            nc.sync.dma_start(gam[:], gam_d[0, :, None])
            bet = sb.tile([64, 1], F32)
            nc.sync.dma_start(bet[:], bet_d[0, :, None])
            scb = sb.tile([64, 2], F32)
            nc.vector.tensor_tensor(out=scb[:, 0:1], in0=rin[:], in1=gam[:],
                                    op=mybir.AluOpType.mult)
            nc.vector.tensor_tensor(out=scb[:, 1:2], in0=me[:, 0:1],
                                    in1=scb[:, 0:1],
                                    op=mybir.AluOpType.mult)
            nc.vector.tensor_tensor(out=scb[:, 1:2], in0=bet[:],
                                    in1=scb[:, 1:2],
                                    op=mybir.AluOpType.subtract)

            # broadcast coefficients along partitions and free dim
            pT = ps.tile([128, 128], F32, tag="psS", space="PSUM")
            nc.tensor.transpose(out=pT[0:1, 0:64], in_=scb[:, 0:1],
                                identity=ident[0:64, 0:64])
            nc.tensor.transpose(out=pT[0:1, 64:128], in_=scb[:, 1:2],
                                identity=ident[0:64, 0:64])
            sr = sb.tile([1, 128], F32)
            nc.vector.tensor_copy(sr[:], pT[0:1, 0:128])
            onesrow = sb.tile([1, 128], F32)
            nc.gpsimd.memset(onesrow[:], 1.0)
            pB = ps.tile([128, 128], F32, tag="psB", space="PSUM")
            nc.tensor.matmul(out=pB[:, 0:64], lhsT=onesrow[:],
                             rhs=sr[:, 0:64], start=True, stop=True)
            nc.tensor.matmul(out=pB[:, 64:128], lhsT=onesrow[:],
                             rhs=sr[:, 64:128], start=True, stop=True)
            SB64 = sb.tile([128, 128], BF16)
            nc.vector.tensor_copy(SB64[:], pB[:])
            S512 = sb.tile([128, 512], BF16)
            B512 = sb.tile([128, 512], BF16)
            for r in range(8):
                nc.vector.tensor_copy(S512[:, r * 64:(r + 1) * 64],
                                      SB64[:, 0:64])
                nc.vector.tensor_copy(B512[:, r * 64:(r + 1) * 64],
                                      SB64[:, 64:128])

            # ---- apply lrelu(h*s + b) in place, then write y ----
            CH = [(j * 512, min(512, NSTAT * D - j * 512))
                  for j in range((NSTAT * D + 511) // 512)]
            yv = y_d[0:24832, :].rearrange("(g two p) c -> two p g c",
                                           two=2, p=128)
            for pi, hs in ((0, hs_e), (1, hs_o)):
                hf = hs[:].rearrange("p g d -> p (g d)")
                for ci, (o, ln) in enumerate(CH):
                    t = io.tile([128, 512], BF16, tag="ap")
                    nc.vector.tensor_tensor(out=t[:, 0:ln], in0=hf[:, o:o + ln],
                                            in1=S512[:, 0:ln],
                                            op=mybir.AluOpType.mult)
                    nc.vector.tensor_tensor(out=t[:, 0:ln], in0=t[:, 0:ln],
                                            in1=B512[:, 0:ln],
                                            op=mybir.AluOpType.add)
                    yst = io.tile([128, 512], F32, tag="yst")
                    nc.scalar.activation(yst[:, 0:ln], t[:, 0:ln],
                                         mybir.ActivationFunctionType.Lrelu,
                                         bias=0.0, scale=1.0, alpha=NEG)
                    ystv = yst[:].rearrange("p (g d) -> p g d", d=D)
                    g0, g1 = o // D, (o + ln) // D
                    if g1 <= 97:
                        nc.sync.dma_start(yv[pi][:, g0:g1, :],
                                          ystv[:, 0:g1 - g0, :])
                    else:
                        nc.sync.dma_start(yv[pi][:, g0:97, :],
                                          ystv[:, 0:97 - g0, :])
                        if pi == 0:
                            nc.sync.dma_start(
                                y_d[24832:24960, :].rearrange("p c -> p c"),
                                ystv[:, 97 - g0, :])
                        else:
                            nc.sync.dma_start(
                                y_d[24960:25000, :].rearrange("p c -> p c"),
                                ystv[0:40, 97 - g0, :])

    nc.compile()
    return nc


_CACHE = {}


def build(nbr):
    nbr = np.asarray(nbr)
    key = nbr.tobytes()[:4096] + nbr.tobytes()[-4096:]
    if key in _CACHE:
        return _CACHE[key]
    plan, GT, XL, grp, gslab, sslab, xsel, perm = _prep_host(
        np.asarray(nbr, np.int64))
    nc = _build_program(plan, GT, XL, grp)
    _CACHE[key] = (nc, gslab, sslab, xsel, perm, XL)
    return _CACHE[key]


def make_in_maps(x, W, gamma, beta, gslab, sslab, xsel, XL):
    x = np.ascontiguousarray(np.asarray(x, np.float32))
    W = np.asarray(W, np.float32)
    import ml_dtypes
    W2 = np.zeros((128, K * 128), ml_dtypes.bfloat16)
    for k in range(K):
        W2[0:D, k * 128:k * 128 + D] = W[k]
        W2[D:128, k * 128 + D:(k + 1) * 128] = W[k]
    gamma = np.asarray(gamma, np.float32).reshape(1, D)
    beta = np.asarray(beta, np.float32).reshape(1, D)
    in_maps = []
    for c in range(C):
        xc = np.zeros((XL, D), np.float32)
        xc[:len(xsel[c])] = x[xsel[c]]
        xcb = np.zeros((25088, D), ml_dtypes.bfloat16)
        xcb[:V] = xc[:V]
        in_maps.append({
            "xc_d": xc,
            "xcb_d": xcb,
            "W2_d": W2,
            "gam_d": gamma,
            "bet_d": beta,
            "gi_d": gslab[c],
            "si_d": sslab[c],
        })
    return in_maps


def kernel(x, W, gamma, beta, nbr):
    nc, gslab, sslab, xsel, perm, XL = build(nbr)
    in_maps = make_in_maps(x, W, gamma, beta, gslab, sslab, xsel, XL)
    res = bass_utils.run_bass_kernel_spmd(nc, in_maps, core_ids=list(range(C)))
    y_ranked = np.concatenate([res.results[c]["y_d"] for c in range(C)], axis=0)
    y = np.empty_like(y_ranked)
    y[perm] = y_ranked
    return y


# revision 32
# speedup vs baseline: 1.3485x; 1.3485x over previous
"""Sparse transposed-conv (27-tap gather-GEMM) + BatchNorm + LeakyReLU on 8 TRN2 cores.

Strategy (component-sharded, SBUF-resident accumulator, bf16 compute):
  - Host: connected components of the neighbor graph are balanced across
    cores (largest-first bin packing) and made contiguous by a global
    reorder; each core owns 25000 voxels plus a small halo of
    cross-boundary source rows, so every gather is core-local
    (< 32768 rows -> single int16 window, one dma_gather per tap).
  - Device, per core: per tap dma_gather fp32 x rows -> PE-transpose
    128x128 chunks to channel-major (cast bf16) -> one bf16 matmul per
    512 columns against a block-diag [[Wk,0],[0,Wk]] stationary ->
    PE-transpose back -> dma_scatter_add into a bf16 SBUF-resident h
    accumulator (parity-split CCE add; v -> parity (v>>7)&1,
    partition v%128, group v>>8). Pads gather a guaranteed-zero x row
    and scatter into trash groups >= TR0.
  - h is split over THREE accumulator pairs (even taps / odd taps /
    center) merged by DVE at the end: scatter-adds to one tile pair form
    a WAW chain that serializes on HW, so independent pairs double the
    scatter overlap. Center accumulates with strided DVE adds (no
    scatter). Gathers/scatters spread over 4 SWDGE queues with a
    queue-aware patch of Tile's DMASW lane rotation.
  - BN mean: per-channel sums fall out of the matmul pipeline for free
    (accum_out on the PSUM->SBUF copy, folded with a [I;I] matmul).
    BN var: ACT squares h, DVE reduces per channel, ones-matmul folds
    partitions; [64,2] AllReduce over 8 cores; y = lrelu(h*s+b) applied
    chunkwise (DVE affine + ACT lrelu) and DMA'd out per chunk.
"""
import numpy as np

import concourse.bass as bass
import concourse.mybir as mybir
import concourse.bacc as bacc
import concourse.tile as tile
from concourse import bass_utils
from concourse.masks import make_identity

# Tile assigns SWDGE completion-sem lanes round-robin over Pool DMAs,
# assuming FIFO completion within a lane. With multiple SWDGE queues that
# assumption breaks (cross-queue reordering on one counting lane lets a
# consumer's wait_ge pass early). Make the lane a function of the queue:
# 8 lanes / 4 queues = 2 lanes per queue, each lane fed by exactly one
# queue (FIFO ring) so counting waits stay sound.
import concourse.tile_sem_assignment as _tsa
from concourse.tile_scheduler import DMAInst as _DMAInst
import concourse.bass_isa as _bass_isa

_orig_assign_tick = _tsa.TileClockTick._assign_tick


def _qaware_assign_tick(self, inst):
    if (isinstance(inst, _DMAInst)
            and not isinstance(inst, _bass_isa.UserSyncedRemoteDMADescs)
            and inst.engine == mybir.EngineType.Pool):
        q = int(getattr(inst, "queue_num", 0) or 0)
        ctr = getattr(self, "_qlane_ctr", None)
        if ctr is None:
            ctr = self._qlane_ctr = {}
        self.next_sw_dma_idx = q * 2 + ctr.get(q, 0) % 2
        ctr[q] = ctr.get(q, 0) + 1
    return _orig_assign_tick(self, inst)


if _tsa.TileClockTick._assign_tick is not _qaware_assign_tick:
    _tsa.TileClockTick._assign_tick = _qaware_assign_tick

N = 200000
C = 8
V = N // C          # 25000 voxels per core
D = 64
K = 27
KC = 13             # center tap (identity map)
GRP = 100           # h groups per parity buffer (covers v < 25600)
NSTAT = 98          # groups included in BN stats / y (v < 25088)
TR0 = 25088         # trash dst base (groups 98, 99)
EPS = 1e-5
NEG = 0.01
F32 = mybir.dt.float32
BF16 = mybir.dt.bfloat16
I16 = mybir.dt.int16


def _r128(n):
    return (n + 127) & ~127


def _r256(n):
    return (n + 255) & ~255


def _pack16(slab, col0, vals):
    """Place index list (len mult of 16) at int16-slab columns col0.., wrapped
    [i%16, i//16] and replicated to all 8 Q7 core partition groups."""
    w = vals.reshape(-1, 16).T
    L16 = w.shape[1]
    for r in range(8):
        slab[r * 16:(r + 1) * 16, col0:col0 + L16] = w
    return col0 + L16


def _components(nbr):
    import scipy.sparse as sp
    import scipy.sparse.csgraph as csg
    Kk, n = nbr.shape
    src = np.repeat(np.arange(n), Kk)
    dst = nbr.T.ravel()
    m = dst >= 0
    A = sp.coo_matrix((np.ones(m.sum(), np.int8), (src[m], dst[m])),
                      shape=(n, n))
    _, lab = csg.connected_components(A, directed=False)
    return lab


def _prep_host(nbr):
    """Component-contiguous reorder + per-core compacted local kernel maps."""
    nbr = np.asarray(nbr, np.int64)
    lab = _components(nbr)
    # balance whole components across cores (largest-first, least-loaded)
    import heapq
    sizes = np.bincount(lab)
    assign = np.empty(len(sizes), np.int64)
    heap = [(0, c) for c in range(C)]
    heapq.heapify(heap)
    for comp in np.argsort(sizes)[::-1]:
        load, c = heapq.heappop(heap)
        assign[comp] = c
        heapq.heappush(heap, (load + int(sizes[comp]), c))
    perm = np.lexsort((lab, assign[lab]))      # rank -> orig
    rank = np.empty(N, np.int64)
    rank[perm] = np.arange(N)                  # orig -> rank

    # per-(core, tap) local (src, dst) pair lists
    ks = [k for k in range(K) if k != KC]
    nbr_r = np.where(nbr >= 0, rank[np.clip(nbr, 0, None)], -1)  # in rank space
    nbr_r = nbr_r[:, perm]                     # column r: dst rank r
    lists = {}
    halos = []
    for c in range(C):
        lo, hi = c * V, (c + 1) * V
        ext = set()
        per_k = {}
        for k in ks:
            srcs = nbr_r[k, lo:hi]
            valid = np.nonzero(srcs >= 0)[0]
            s = srcs[valid]
            per_k[k] = (s, valid)
            out = s[(s < lo) | (s >= hi)]
            ext.update(out.tolist())
        halo = np.sort(np.fromiter(ext, np.int64, len(ext)))
        halos.append(halo)
        for k in ks:
            s, valid = per_k[k]
            inr = (s >= lo) & (s < hi)
            loc = np.where(inr, s - lo, V + np.searchsorted(halo, s))
            o = np.argsort(loc, kind="stable")
            lists[(c, k)] = (loc[o].astype(np.int16), valid[o].astype(np.int16))

    XL = _r128(V + max(len(h) for h in halos) + 1)
    assert XL <= 32768, XL

    NK = {k: max(256, _r256(max(len(lists[(c, k)][0]) for c in range(C))))
          for k in ks}
    GT = sum(NK.values())
    max_npad = max(NK[k] - len(lists[(c, k)][0])
                   for k in ks for c in range(C))
    grp = max(GRP, -(-(TR0 + max_npad) // 256))
    gslab = np.zeros((C, 128, GT // 16), np.int16)
    sslab = np.zeros((C, 128, GT // 16), np.int16)
    plan = []
    koff = 0
    for k in ks:
        plan.append((k, NK[k], koff))
        for c in range(C):
            g, s = lists[(c, k)]
            L = NK[k]
            gp = np.full(L, XL - 1, np.int16)
            gp[:len(g)] = g
            sp_ = np.empty(L, np.int16)
            sp_[:len(s)] = s
            npad = L - len(s)
            assert npad <= 256 * grp - TR0, (npad, grp)
            if npad:
                sp_[len(s):] = TR0 + np.arange(npad, dtype=np.int16)
            _pack16(gslab[c], koff // 16, gp)
            _pack16(sslab[c], koff // 16, sp_)
        koff += NK[k]

    xsel = []
    for c in range(C):
        sel = np.concatenate([perm[c * V:(c + 1) * V], perm[halos[c]]])
        xsel.append(sel)
    return plan, GT, XL, grp, gslab, sslab, xsel, perm


def _build_program(plan, GT, XL, grp):
    nc = bacc.Bacc("TRN2", target_bir_lowering=False, debug=False,
                   num_devices=C, num_swdge_queues=4)

    xc_d = nc.dram_tensor("xc_d", [XL, D], F32, kind="ExternalInput")
    xcb_d = nc.dram_tensor("xcb_d", [25088, D], BF16, kind="ExternalInput")
    W2_d = nc.dram_tensor("W2_d", [128, K * 128], BF16, kind="ExternalInput")
    gam_d = nc.dram_tensor("gam_d", [1, D], F32, kind="ExternalInput")
    bet_d = nc.dram_tensor("bet_d", [1, D], F32, kind="ExternalInput")
    gi_d = nc.dram_tensor("gi_d", [128, GT // 16], I16, kind="ExternalInput")
    si_d = nc.dram_tensor("si_d", [128, GT // 16], I16, kind="ExternalInput")
    y_d = nc.dram_tensor("y_d", [V, D], F32, kind="ExternalOutput")
    import os as _os
    _dbg = _os.environ.get("KERNEL_DEBUG_H")
    if _dbg:
        hdbg_d = nc.dram_tensor("hdbg_d", [2, 128, grp * D], BF16,
                                kind="ExternalOutput")

    NKmax = max(nk for _, nk, _ in plan)

    with tile.TileContext(nc) as tc:
        with tc.tile_pool(name="sb", bufs=1) as sb, \
             tc.tile_pool(name="io", bufs=3) as io, \
             tc.tile_pool(name="ps", bufs=2, space="PSUM") as ps, \
             tc.tile_pool(name="dram", bufs=1, space="DRAM") as dram:

            ident = sb.tile([128, 128], F32)
            make_identity(nc, ident[:])
            ident_b = sb.tile([128, 128], BF16)
            nc.vector.tensor_copy(ident_b[:], ident[:])
            gi_t = sb.tile([128, GT // 16], I16)
            nc.sync.dma_start(gi_t[:], gi_d[:, :])
            si_t = sb.tile([128, GT // 16], I16)
            nc.sync.dma_start(si_t[:], si_d[:, :])

            W2 = sb.tile([128, K * 128], BF16)
            nc.sync.dma_start(W2[:], W2_d[:, :])

            hs_e = sb.tile([128, grp, D], BF16)
            hs_o = sb.tile([128, grp, D], BF16)
            hs_e2 = sb.tile([128, grp, D], BF16)
            hs_o2 = sb.tile([128, grp, D], BF16)
            hs_ec = sb.tile([128, grp, D], BF16)
            hs_oc = sb.tile([128, grp, D], BF16)
            hs_e3 = sb.tile([128, grp, D], BF16)
            hs_o3 = sb.tile([128, grp, D], BF16)
            nc.gpsimd.memset(hs_e3[:], 0.0)
            nc.gpsimd.memset(hs_o3[:], 0.0)
            nc.gpsimd.memset(hs_e[:], 0.0)
            nc.gpsimd.memset(hs_o[:], 0.0)
            nc.gpsimd.memset(hs_e2[:], 0.0)
            nc.gpsimd.memset(hs_o2[:], 0.0)
            nc.gpsimd.memset(hs_ec[:], 0.0)
            nc.gpsimd.memset(hs_oc[:], 0.0)

            acc = sb.tile([128, 128], F32)
            nc.gpsimd.memset(acc[:], 0.0)
            acc_n = [0]

            def mid(gsrc, kk, cols, sout, scol, bf=False):
                """gsrc[:, cols] (2-voxel 128-col chunks) -> transpose -> MM
                W2[kk] -> transpose back -> sout[:, scol:scol+128*len(cols)].
                The PSUM->SBUF copy also bank-sums columns into acc (per
                (slot,ch) partition) for the BN mean."""
                gw = len(cols) * 128
                pa = ps.tile([128, 512], BF16 if bf else F32, tag="psA",
                             space="PSUM")
                idt = ident_b if bf else ident
                for j, cj in enumerate(cols):
                    nc.tensor.transpose(
                        out=pa[:, j * 128:(j + 1) * 128],
                        in_=gsrc[:, cj * 128:(cj + 1) * 128], identity=idt[:])
                ct = io.tile([128, 512], BF16, tag="ct")
                nc.scalar.activation(ct[:, :gw], pa[:, :gw],
                                     mybir.ActivationFunctionType.Copy,
                                     bias=0.0)
                pb = ps.tile([128, 512], F32, tag="psB", space="PSUM")
                nc.tensor.matmul(out=pb[:, :gw],
                                 lhsT=W2[:, kk * 128:(kk + 1) * 128],
                                 rhs=ct[:, :gw], start=True, stop=True)
                hb = io.tile([128, 512], BF16, tag="hb")
                ai = acc_n[0]
                acc_n[0] += 1
                nc.vector.tensor_scalar(out=hb[:, :gw], in0=pb[:, :gw],
                                        scalar1=0.0, scalar2=0.0,
                                        op0=mybir.AluOpType.add,
                                        op1=mybir.AluOpType.add,
                                        accum_out=acc[:, ai:ai + 1])
                pc = ps.tile([128, 512], BF16, tag="psC", space="PSUM")
                for j in range(len(cols)):
                    nc.tensor.transpose(
                        out=pc[:, j * 128:(j + 1) * 128],
                        in_=hb[:, j * 128:(j + 1) * 128], identity=ident_b[:])
                nc.scalar.activation(sout[:, scol:scol + gw], pc[:, :gw],
                                     mybir.ActivationFunctionType.Copy,
                                     bias=0.0)

            # ---- center tap: dense, strided-copied into h ----
            def center_group(g):
                rows = 512 if g == 24 else 1024
                nm = rows // 128
                xg = io.tile([128, 8, D], BF16, tag="xg")
                xv = xcb_d[g * 1024:g * 1024 + rows, :].rearrange(
                    "(m p) c -> p m c", p=128)
                nc.sync.dma_start(xg[:, 0:nm, :], xv)
                xgf = xg[:].rearrange("p m d -> p (m d)")
                cs = io.tile([128, 8, D], BF16, tag="cs")
                csf = cs[:].rearrange("p m d -> p (m d)")
                mid(xgf, KC, list(range(nm // 2)), csf, 0, bf=True)
                def acc(dst, src):
                    nc.vector.tensor_tensor(out=dst, in0=dst, in1=src,
                                            op=mybir.AluOpType.add)
                if g < 24:
                    acc(hs_ec[:, 4 * g:4 * g + 4, :], cs[:, 0:8:2, :])
                    acc(hs_oc[:, 4 * g:4 * g + 4, :], cs[:, 1:8:2, :])
                else:
                    acc(hs_ec[:, 96:98, :], cs[:, 0:4:2, :])
                    acc(hs_oc[:, 96:97, :], cs[:, 1:2, :])
                    acc(hs_oc[0:40, 97:98, :], cs[0:40, 3:4, :])

            # ---- sparse taps (depth-2 software pipeline) ----
            pdma_ctr = [0]

            def nextq():
                q = pdma_ctr[0] % 4
                pdma_ctr[0] += 1
                return q

            def tap_gather(ki):
                k, NKk, koff = plan[ki]
                gb = io.tile([128, NKmax // 128, D], F32, tag="gb")
                nc.gpsimd.dma_gather(
                    out_ap=gb[:, 0:NKk // 128, :],
                    in_ap=xc_d[:, :],
                    idxs_ap=gi_t[:, koff // 16:(koff + NKk) // 16],
                    num_idxs=NKk, num_idxs_reg=NKk, elem_size=D,
                    single_packet=False, queue_num=nextq())
                return gb

            def tap_compute(ki, gb):
                k, NKk, koff = plan[ki]
                gbf = gb[:].rearrange("p m d -> p (m d)")
                sk = io.tile([128, NKmax // 128, D], BF16, tag="sk")
                skf = sk[:].rearrange("p m d -> p (m d)")
                nch = NKk // 256
                for c0 in range(0, nch, 4):
                    cols = list(range(c0, min(c0 + 4, nch)))
                    mid(gbf, k, cols, skf, c0 * 128)
                he, ho = ((hs_e, hs_o), (hs_e2, hs_o2),
                          (hs_e3, hs_o3))[ki % 3]
                nc.gpsimd.dma_scatter_add(
                    out_ap=he[:], out_ap_other=ho[:],
                    in_ap=sk[:, 0:NKk // 128, :],
                    idxs_ap=si_t[:, koff // 16:(koff + NKk) // 16],
                    num_idxs=NKk, num_idxs_reg=NKk, elem_size=D,
                    single_packet=False, queue_num=nextq(),
                    sbuf_tokens_per_rank=128, parity_reg=0)

            pend = []
            cg = 0
            for ki in range(len(plan)):
                pend.append((ki, tap_gather(ki)))
                if cg < 25:
                    center_group(cg)
                    cg += 1
                if len(pend) > 2:
                    kj, gbj = pend.pop(0)
                    tap_compute(kj, gbj)
            while cg < 25:
                center_group(cg)
                cg += 1
            for kj, gbj in pend:
                tap_compute(kj, gbj)

            if _dbg:
                nc.sync.dma_start(hdbg_d[0, :, :],
                                  hs_e[:].rearrange("p g d -> p (g d)"))
                nc.sync.dma_start(hdbg_d[1, :, :],
                                  hs_o[:].rearrange("p g d -> p (g d)"))
            # merge the extra accumulator pairs
            for dst_t, src_t in ((hs_e, hs_e2), (hs_o, hs_o2),
                                 (hs_e, hs_e3), (hs_o, hs_o3),
                                 (hs_e, hs_ec), (hs_o, hs_oc)):
                nc.vector.tensor_tensor(
                    out=dst_t[:].rearrange("p g d -> p (g d)"),
                    in0=dst_t[:].rearrange("p g d -> p (g d)"),
                    in1=src_t[:].rearrange("p g d -> p (g d)"),
                    op=mybir.AluOpType.add)
            # ---- BN stats: mean from acc, sumsq from SBUF h ----
            q_pch = sb.tile([128, D], F32)
            q_tmp = sb.tile([128, D], F32)
            for pi, hs in ((0, hs_e), (1, hs_o)):
                qo = q_pch if pi == 0 else q_tmp
                sq_t = hs_e2 if pi == 0 else hs_o2
                sqf = sq_t[:, 0:NSTAT, :].rearrange("p g d -> p (g d)")
                hf = hs[:, 0:NSTAT, :].rearrange("p g d -> p (g d)")
                nc.scalar.activation(sqf, hf,
                                     mybir.ActivationFunctionType.Square,
                                     bias=0.0, scale=1.0)
                qview = sq_t[:, 0:NSTAT, :].rearrange("p g d -> p d g")
                nc.vector.tensor_reduce(out=qo[:], in_=qview,
                                        axis=mybir.AxisListType.X,
                                        op=mybir.AluOpType.add)
            nc.vector.tensor_tensor(out=q_pch[:], in0=q_pch[:], in1=q_tmp[:],
                                    op=mybir.AluOpType.add)
            acc_red = sb.tile([128, 1], F32)
            nc.vector.tensor_reduce(out=acc_red[:], in_=acc[:],
                                    axis=mybir.AxisListType.X,
                                    op=mybir.AluOpType.add)
            fold = sb.tile([128, D], F32)
            nc.vector.tensor_copy(fold[0:64, :], ident[0:64, 0:64])
            nc.vector.tensor_copy(fold[64:128, :], ident[0:64, 0:64])
            ones1 = sb.tile([128, 1], F32)
            nc.gpsimd.memset(ones1[:], 1.0)
            pS = ps.tile([64, 2], F32, tag="psS", space="PSUM")
            nc.tensor.matmul(out=pS[:, 0:1], lhsT=fold[:], rhs=acc_red[:],
                             start=True, stop=True)
            nc.tensor.matmul(out=pS[:, 1:2], lhsT=q_pch[:], rhs=ones1[:],
                             start=True, stop=True)
            sq64 = sb.tile([64, 2], F32)
            nc.vector.tensor_copy(sq64[:], pS[:])

            cc_in = dram.tile([64, 2], F32)
            cc_out = dram.tile([64, 2], F32)
            nc.sync.dma_start(cc_in[:], sq64[:])
            import os as _os
            if _os.environ.get("KERNEL_SIM_NOCC"):
                nc.sync.dma_start(cc_out[:], cc_in[:])
            else:
                nc.gpsimd.collective_compute(
                    "AllReduce", mybir.AluOpType.add,
                    replica_groups=[list(range(C))],
                    ins=[cc_in.opt()], outs=[cc_out.opt()])
            g2 = sb.tile([64, 2], F32)
            nc.sync.dma_start(g2[:], cc_out[:])

            # per-channel BN coefficients (channel-major on 64 partitions)
            me = sb.tile([64, 2], F32)
            nc.vector.tensor_scalar_mul(me[:], g2[:], 1.0 / N)  # [mean, Eh2]
            v1 = sb.tile([64, 1], F32)
            nc.vector.tensor_tensor(out=v1[:], in0=me[:, 0:1], in1=me[:, 0:1],
                                    op=mybir.AluOpType.mult)
            nc.vector.tensor_tensor(out=v1[:], in0=me[:, 1:2], in1=v1[:],
                                    op=mybir.AluOpType.subtract)
            eps_t = sb.tile([64, 1], F32)
            nc.gpsimd.memset(eps_t[:], EPS)
            std = sb.tile([64, 1], F32)
            nc.scalar.activation(std[:], v1[:],
                                 mybir.ActivationFunctionType.Sqrt,
                                 bias=eps_t[:])
            rin = sb.tile([64, 1], F32)
            nc.vector.reciprocal(rin[:], std[:])
            gam = sb.tile([64, 1], F32)
            nc.sync.dma_start(gam[:], gam_d[0, :, None])
            bet = sb.tile([64, 1], F32)
            nc.sync.dma_start(bet[:], bet_d[0, :, None])
            scb = sb.tile([64, 2], F32)
            nc.vector.tensor_tensor(out=scb[:, 0:1], in0=rin[:], in1=gam[:],
                                    op=mybir.AluOpType.mult)
            nc.vector.tensor_tensor(out=scb[:, 1:2], in0=me[:, 0:1],
                                    in1=scb[:, 0:1],
                                    op=mybir.AluOpType.mult)
            nc.vector.tensor_tensor(out=scb[:, 1:2], in0=bet[:],
                                    in1=scb[:, 1:2],
                                    op=mybir.AluOpType.subtract)

            # broadcast coefficients along partitions and free dim
            pT = ps.tile([128, 128], F32, tag="psS", space="PSUM")
            nc.tensor.transpose(out=pT[0:1, 0:64], in_=scb[:, 0:1],
                                identity=ident[0:64, 0:64])
            nc.tensor.transpose(out=pT[0:1, 64:128], in_=scb[:, 1:2],
                                identity=ident[0:64, 0:64])
            sr = sb.tile([1, 128], F32)
            nc.vector.tensor_copy(sr[:], pT[0:1, 0:128])
            onesrow = sb.tile([1, 128], F32)
            nc.gpsimd.memset(onesrow[:], 1.0)
            pB = ps.tile([128, 128], F32, tag="psB", space="PSUM")
            nc.tensor.matmul(out=pB[:, 0:64], lhsT=onesrow[:],
                             rhs=sr[:, 0:64], start=True, stop=True)
            nc.tensor.matmul(out=pB[:, 64:128], lhsT=onesrow[:],
                             rhs=sr[:, 64:128], start=True, stop=True)
            SB64 = sb.tile([128, 128], BF16)
            nc.vector.tensor_copy(SB64[:], pB[:])
            S512 = sb.tile([128, 512], BF16)
            B512 = sb.tile([128, 512], BF16)
            for r in range(8):
                nc.vector.tensor_copy(S512[:, r * 64:(r + 1) * 64],
                                      SB64[:, 0:64])
                nc.vector.tensor_copy(B512[:, r * 64:(r + 1) * 64],
                                      SB64[:, 64:128])

            # ---- apply lrelu(h*s + b) in place, then write y ----
            CH = [(j * 512, min(512, NSTAT * D - j * 512))
                  for j in range((NSTAT * D + 511) // 512)]
            yv = y_d[0:24832, :].rearrange("(g two p) c -> two p g c",
                                           two=2, p=128)
            for pi, hs in ((0, hs_e), (1, hs_o)):
                hf = hs[:].rearrange("p g d -> p (g d)")
                for ci, (o, ln) in enumerate(CH):
                    t = io.tile([128, 512], BF16, tag="ap")
                    nc.vector.tensor_tensor(out=t[:, 0:ln], in0=hf[:, o:o + ln],
                                            in1=S512[:, 0:ln],
                                            op=mybir.AluOpType.mult)
                    nc.vector.tensor_tensor(out=t[:, 0:ln], in0=t[:, 0:ln],
                                            in1=B512[:, 0:ln],
                                            op=mybir.AluOpType.add)
                    yst = io.tile([128, 512], F32, tag="yst")
                    nc.scalar.activation(yst[:, 0:ln], t[:, 0:ln],
                                         mybir.ActivationFunctionType.Lrelu,
                                         bias=0.0, scale=1.0, alpha=NEG)
                    ystv = yst[:].rearrange("p (g d) -> p g d", d=D)
                    g0, g1 = o // D, (o + ln) // D
                    if g1 <= 97:
                        nc.sync.dma_start(yv[pi][:, g0:g1, :],
                                          ystv[:, 0:g1 - g0, :])
                    else:
                        nc.sync.dma_start(yv[pi][:, g0:97, :],
                                          ystv[:, 0:97 - g0, :])
                        if pi == 0:
                            nc.sync.dma_start(
                                y_d[24832:24960, :].rearrange("p c -> p c"),
                                ystv[:, 97 - g0, :])
                        else:
                            nc.sync.dma_start(
                                y_d[24960:25000, :].rearrange("p c -> p c"),
                                ystv[0:40, 97 - g0, :])

    nc.compile()
    return nc


_CACHE = {}


def build(nbr):
    nbr = np.asarray(nbr)
    key = nbr.tobytes()[:4096] + nbr.tobytes()[-4096:]
    if key in _CACHE:
        return _CACHE[key]
    plan, GT, XL, grp, gslab, sslab, xsel, perm = _prep_host(
        np.asarray(nbr, np.int64))
    nc = _build_program(plan, GT, XL, grp)
    _CACHE[key] = (nc, gslab, sslab, xsel, perm, XL)
    return _CACHE[key]


def make_in_maps(x, W, gamma, beta, gslab, sslab, xsel, XL):
    x = np.ascontiguousarray(np.asarray(x, np.float32))
    W = np.asarray(W, np.float32)
    import ml_dtypes
    W2 = np.zeros((128, K * 128), ml_dtypes.bfloat16)
    for k in range(K):
        W2[0:D, k * 128:k * 128 + D] = W[k]
        W2[D:128, k * 128 + D:(k + 1) * 128] = W[k]
    gamma = np.asarray(gamma, np.float32).reshape(1, D)
    beta = np.asarray(beta, np.float32).reshape(1, D)
    in_maps = []
    for c in range(C):
        xc = np.zeros((XL, D), np.float32)
        xc[:len(xsel[c])] = x[xsel[c]]
        xcb = np.zeros((25088, D), ml_dtypes.bfloat16)
        xcb[:V] = xc[:V]
        in_maps.append({
            "xc_d": xc,
            "xcb_d": xcb,
            "W2_d": W2,
            "gam_d": gamma,
            "bet_d": beta,
            "gi_d": gslab[c],
            "si_d": sslab[c],
        })
    return in_maps


def kernel(x, W, gamma, beta, nbr):
    nc, gslab, sslab, xsel, perm, XL = build(nbr)
    in_maps = make_in_maps(x, W, gamma, beta, gslab, sslab, xsel, XL)
    res = bass_utils.run_bass_kernel_spmd(nc, in_maps, core_ids=list(range(C)))
    y_ranked = np.concatenate([res.results[c]["y_d"] for c in range(C)], axis=0)
    y = np.empty_like(y_ranked)
    y[perm] = y_ranked
    return y
